# revision 3
# baseline (speedup 1.0000x reference)
"""CARAFE-downsample Trainium2 kernel (B=8, C=256, H=W=128, K=5, S=2, M=64).

Sharding: data-parallel over batch B across 8 NeuronCores (one sample per core).

Per-core pipeline (all heavy compute on TensorE, bf16):
  1. SWDGE cast-DMA  x[b] fp32 DRAM -> bf16 SBUF, channel-major [2x128c, 16384pix]
  2. xbar DMA-transpose -> xT pixel-major tiles [128 iw, 131 rows(+3 zero pad), 256 c]
  3. compress 1x1 conv (C=256 -> M=64) as 2-chunk accumulating matmuls,
     bias fused in PSUM->SBUF copy, written into zero-padded ker1 [64, 130, 130]
  4. encoder 3x3 stride-2 conv (M=64 -> 25) as 9 accumulating matmuls with
     strided gather APs on ker1
  5. PE-transpose ker2 [25, 4096] -> [4096, 25] blocks; softmax along free dim
  6. per output-row-pair P: gpsimd local_scatter builds A^T[p, (ridx, iw)]
     (scattered softmax weights, constant index map), xbar DMA-transpose -> A
  7. out[c, p] = sum_q xT[q, c]^T A[q, p]: 7 accumulating matmuls per (P, c-chunk)
  8. PSUM -> SBUF staging -> 512 KB DMAs to DRAM (channel-major, fp32)
"""

import sys

if "/opt/trn_rl_repo" not in sys.path:
    sys.path.insert(0, "/opt/trn_rl_repo")

import numpy as np
import ml_dtypes

import concourse.bacc as bacc
import concourse.tile as tile
from concourse import mybir
from concourse.bass_utils import run_bass_kernel_spmd

F32 = mybir.dt.float32
BF16 = mybir.dt.float16
I16 = mybir.dt.int16

B, C, H, W = 8, 256, 128, 128
M = 64          # compressed channels
K = 5           # carafe kernel size
S = 2           # stride
KK = K * K      # 25
nH, nW = H // S, W // S          # 64, 64
NPIX = H * W                     # 16384
NOPIX = nH * nW                  # 4096
NPAIR = nH // 2                  # 32 output-row pairs
QCH = 7                          # input rows per pair band (4P-2 .. 4P+4)
APITCH = 144                     # free pitch of A chunks (non-mergeable, 32B-aligned)
NIDX = 26                        # local_scatter num_idxs (25 taps + pad)
NROWT = H + 3                    # xT row tiles: 2 top pad + 128 + 1 bottom pad


def _exp_taps():
    """Constant per-partition scatter index map: idx[p, k] -> (ridx, iw) slot."""
    idx = np.full((128, NIDX), -1, dtype=np.int16)
    for p in range(128):
        doh, ow = p // nW, p % nW
        for k in range(KK):
            i, j = k // K, k % K
            iw = 2 * ow + j - 2
            if 0 <= iw < W:
                idx[p, k] = (2 * doh + i) * W + iw
    return idx


def _build_program():
    nc = bacc.Bacc("TRN2", target_bir_lowering=False, debug=False, num_devices=8)

    x_d = nc.dram_tensor("x", [C, NPIX], F32, kind="ExternalInput")
    w1_d = nc.dram_tensor("w1t", [128, 2, M], BF16, kind="ExternalInput")
    w2_d = nc.dram_tensor("w2t", [M, 9, KK], BF16, kind="ExternalInput")
    b1_d = nc.dram_tensor("b1c", [M, 1], F32, kind="ExternalInput")
    b2_d = nc.dram_tensor("b2c", [KK, 1], F32, kind="ExternalInput")
    id_d = nc.dram_tensor("ident", [KK, KK], BF16, kind="ExternalInput")
    ix_d = nc.dram_tensor("scidx", [128, NIDX], I16, kind="ExternalInput")
    o_d = nc.dram_tensor("out", [C, NOPIX], F32, kind="ExternalOutput")

    with tile.TileContext(nc) as tc:
        with (
            tc.tile_pool(name="const", bufs=1) as constp,
            tc.tile_pool(name="xq", bufs=2) as xqp,
            tc.tile_pool(name="big", bufs=1) as bigp,
            tc.tile_pool(name="sm", bufs=3) as smp,
            tc.tile_pool(name="ab", bufs=3) as abp,
            tc.tile_pool(name="ost", bufs=2) as ostp,
            tc.tile_pool(name="psA", bufs=3, space="PSUM") as psA,
            tc.tile_pool(name="psB", bufs=3, space="PSUM") as psB,
        ):
            # ---- constants ----
            w1sb = constp.tile([128, 2, M], BF16)
            nc.sync.dma_start(out=w1sb[:], in_=w1_d.ap())
            w2sb = constp.tile([M, 9, KK], BF16)
            nc.sync.dma_start(out=w2sb[:], in_=w2_d.ap())
            b1sb = constp.tile([M, 1], F32)
            nc.sync.dma_start(out=b1sb[:], in_=b1_d.ap())
            b2sb = constp.tile([KK, 1], F32)
            nc.sync.dma_start(out=b2sb[:], in_=b2_d.ap())
            idsb = constp.tile([KK, KK], BF16)
            nc.sync.dma_start(out=idsb[:], in_=id_d.ap())
            ixsb = constp.tile([128, NIDX], I16)
            nc.sync.dma_start(out=ixsb[:], in_=ix_d.ap())

            # ---- persistent tensors ----
            xT = bigp.tile([128, NROWT, C], BF16)       # pixel-major x (padded rows)
            ker1 = bigp.tile([M, H + 2, W + 2], BF16)   # compressed, zero-padded
            ker2 = bigp.tile([KK, NOPIX], BF16)         # encoder logits, channel-major
            esb = bigp.tile([128, NPAIR, KK], F32)      # exp(logits), pixel-major
            ssum = bigp.tile([128, NPAIR], F32)
            rsum = bigp.tile([128, NPAIR], F32)

            # zero pads (row tiles -2,-1,+128 of xT; border of ker1)
            nc.vector.memset(xT[:, 0:2, :], 0.0)
            nc.vector.memset(xT[:, NROWT - 1 : NROWT, :], 0.0)
            nc.vector.memset(ker1[:, 0, :], 0.0)
            nc.vector.memset(ker1[:, H + 1, :], 0.0)
            nc.vector.memset(ker1[:, :, 0], 0.0)
            nc.vector.memset(ker1[:, :, W + 1], 0.0)

            # ---- phase A: load x (cast bf16), transpose to xT, compress conv ----
            for q in range(4):          # quarters of the image (32 rows each)
                xq0 = xqp.tile([128, 4096], BF16, tag="xq0")
                xq1 = xqp.tile([128, 4096], BF16, tag="xq1")
                nc.gpsimd.dma_start(
                    out=xq0[:], in_=x_d.ap()[0:128, q * 4096 : (q + 1) * 4096]
                )
                nc.gpsimd.dma_start(
                    out=xq1[:], in_=x_d.ap()[128:256, q * 4096 : (q + 1) * 4096]
                )
                # batched xbar transpose: out[iw, r, c] = in[c, r*128 + iw]
                nc.sync.dma_start(
                    out=xT[:, 2 + q * 32 : 2 + (q + 1) * 32, 0:128],
                    in_=xq0[:],
                    transpose=True,
                )
                nc.sync.dma_start(
                    out=xT[:, 2 + q * 32 : 2 + (q + 1) * 32, 128:256],
                    in_=xq1[:],
                    transpose=True,
                )
                for blk in range(8):    # 512-pixel (4-row) blocks
                    ps1 = psA.tile([M, 512], F32, tag="mm")
                    nc.tensor.matmul(
                        ps1[:],
                        lhsT=w1sb[:, 0, :],
                        rhs=xq0[:, blk * 512 : (blk + 1) * 512],
                        start=True,
                        stop=False,
                    )
                    nc.tensor.matmul(
                        ps1[:],
                        lhsT=w1sb[:, 1, :],
                        rhs=xq1[:, blk * 512 : (blk + 1) * 512],
                        start=False,
                        stop=True,
                    )
                    r0 = q * 32 + blk * 4
                    nc.vector.tensor_scalar_add(
                        out=ker1[:, 1 + r0 : 1 + r0 + 4, 1 : 1 + W],
                        in0=ps1[:].rearrange("p (r w) -> p r w", r=4),
                        scalar1=b1sb[:],
                    )

            # ---- phase B: encoder 3x3 stride-2 conv ----
            # ker1 viewed [64, 65, 2, 65, 2] to expose stride-2 gathers
            kv = ker1[:].rearrange("p (r s) (w t) -> p r s w t", s=2, t=2)
            for cb in range(8):         # 512-output-pixel (8-oh-row) blocks
                ps2 = psA.tile([KK, 512], F32, tag="mm")
                for tap in range(9):
                    dy, dx = tap // 3, tap % 3
                    m0 = 8 * cb + dy // 2
                    n0 = dx // 2
                    rhs = kv[:, m0 : m0 + 8, dy & 1, n0 : n0 + nW, dx & 1]
                    nc.tensor.matmul(
                        ps2[:],
                        lhsT=w2sb[:, tap, :],
                        rhs=rhs,
                        start=(tap == 0),
                        stop=(tap == 8),
                    )
                nc.vector.tensor_scalar_add(
                    out=ker2[:, cb * 512 : (cb + 1) * 512],
                    in0=ps2[:],
                    scalar1=b2sb[:],
                )

            # ---- phase C: transpose logits to pixel-major, softmax ----
            for pt in range(NPAIR):
                psT = psB.tile([128, KK], BF16, tag="tr")
                nc.tensor.transpose(
                    psT[:], ker2[:, pt * 128 : (pt + 1) * 128], idsb[:]
                )
                nc.scalar.activation(
                    out=esb[:, pt, :],
                    in_=psT[:],
                    func=mybir.ActivationFunctionType.Exp,
                )
            nc.vector.tensor_reduce(
                out=ssum[:],
                in_=esb[:],
                axis=mybir.AxisListType.X,
                op=mybir.AluOpType.add,
            )
            nc.vector.reciprocal(out=rsum[:], in_=ssum[:])

            # ---- phase D: per row-pair scatter + weighted-sum matmuls ----
            for grp in range(4):
                osb0 = ostp.tile([128, 8 * 128], F32, tag="o0")
                osb1 = ostp.tile([128, 8 * 128], F32, tag="o1")
                for pos in range(8):
                    P = grp * 8 + pos
                    wn = smp.tile([128, NIDX], BF16, tag="wn")
                    nc.vector.tensor_scalar_mul(
                        out=wn[:, 0:KK],
                        in0=esb[:, P, :],
                        scalar1=rsum[:, P : P + 1],
                    )
                    nc.vector.memset(wn[:, KK:NIDX], 0.0)
                    at = smp.tile([128, QCH * 128], BF16, tag="at")
                    nc.gpsimd.local_scatter(
                        out_ap=at[:],
                        data_ap=wn[:],
                        idxs_ap=ixsb[:],
                        channels=128,
                        num_elems=QCH * 128,
                        num_idxs=NIDX,
                    )
                    amat = abp.tile([128, QCH, APITCH], BF16, tag="A")
                    nc.sync.dma_start(
                        out=amat[:, :, 0:128], in_=at[:], transpose=True
                    )
                    for cc in range(2):
                        psF = psA.tile([128, 128], F32, tag="mm")
                        for qq in range(QCH):
                            t = 4 * P + qq      # padded xT row-tile index
                            nc.tensor.matmul(
                                psF[:],
                                lhsT=xT[:, t, cc * 128 : (cc + 1) * 128],
                                rhs=amat[:, qq, 0:128],
                                start=(qq == 0),
                                stop=(qq == QCH - 1),
                            )
                        osb = osb0 if cc == 0 else osb1
                        if pos % 2 == 0:
                            nc.scalar.copy(
                                out=osb[:, pos * 128 : (pos + 1) * 128], in_=psF[:]
                            )
                        else:
                            nc.vector.tensor_copy(
                                out=osb[:, pos * 128 : (pos + 1) * 128], in_=psF[:]
                            )
                nc.sync.dma_start(
                    out=o_d.ap()[0:128, grp * 1024 : (grp + 1) * 1024], in_=osb0[:]
                )
                nc.sync.dma_start(
                    out=o_d.ap()[128:256, grp * 1024 : (grp + 1) * 1024], in_=osb1[:]
                )

    nc.compile()
    return nc


_NC = None


def _get_nc():
    global _NC
    if _NC is None:
        _NC = _build_program()
    return _NC


def _host_inputs(w1, b1, w2, b2):
    """Precompute constant / rearranged weight tensors (host-side, numpy)."""
    bf = np.float16
    w1m = np.asarray(w1, np.float32).reshape(M, C)            # [m, c]
    w1t = np.transpose(w1m.reshape(M, 2, 128), (2, 1, 0)).astype(bf)  # [cp, chunk, m]
    w1t = np.ascontiguousarray(w1t)
    w2m = np.asarray(w2, np.float32).reshape(KK, M, 9)        # [k, m, tap]
    w2t = np.ascontiguousarray(np.transpose(w2m, (1, 2, 0))).astype(bf)  # [m, tap, k]
    b1c = np.asarray(b1, np.float32).reshape(M, 1).copy()
    b2c = np.asarray(b2, np.float32).reshape(KK, 1).copy()
    ident = np.eye(KK, dtype=bf)
    scidx = _exp_taps()
    return {
        "w1t": w1t,
        "w2t": w2t,
        "b1c": b1c,
        "b2c": b2c,
        "ident": ident,
        "scidx": scidx,
    }


def kernel(x, w1, b1, w2, b2):
    x = np.asarray(x, np.float32)
    consts = _host_inputs(w1, b1, w2, b2)
    nc = _get_nc()
    in_maps = []
    for b in range(B):
        m = {"x": np.ascontiguousarray(x[b].reshape(C, NPIX))}
        m.update(consts)
        in_maps.append(m)
    res = run_bass_kernel_spmd(nc, in_maps, core_ids=list(range(B)))
    out = np.stack([res.results[i]["out"] for i in range(B)], axis=0)
    return out.reshape(B, C, nH, nW)
